# revision 42
# baseline (speedup 1.0000x reference)
"""GATv2 2-layer GNN message-passing kernel for Trainium2, 8-core SPMD.

Contract: kernel(**inputs) takes the FULL unsharded inputs (as produced by
setup_inputs) and returns the FULL [50000, 128] float32 output.

Strategy (edge/data parallel, dst-range sharded), v3:
- Host: append self-loops, sort edges by dst, give each of the 8 cores an
  equal contiguous dst-node range (6250 nodes = 49 blocks of 128). Within
  each block, edges are split by src-half so the int16 dma_gather indices
  stay < 32768 (two source tables); layer 2 gets its own split/slots since
  the chunked AllGather layout is not monotone in node id. Per-block group
  counts are padded to the max over the 8 cores (not the global max) so one
  SPMD program serves all cores with minimal slot waste.
- Fused per-block pipeline (no DRAM staging): per block, dma_gather calls
  (xl from the src tables, xr from the local dst table, <=1024 idx each)
  land in double-buffered SBUF tiles; compute consumes them directly.
- SWDGE queue assignment is the single biggest HW lever (measured 8.3ms ->
  3.5ms): the 8 DMASW sem lanes are given to SWDGE DMAs in SCHEDULED order
  and each lane is locked to one queue, so build_kernel_auto compiles once,
  reads the scheduled gather order, rebuilds with queue = position % 4, and
  re-verifies the lane->queue map (falling back to a single queue).
- All edge-path tensors are bf16. The gather tables are pre-scaled by
  |att| with columns permuted so each head's positive-att columns come
  first: the per-edge attention dot then reduces to
  e = sum_pos Prelu(z~) - sum_neg Prelu(z~), killing the att multiply.
  Sign fix via 4x-mode tensor_scalar negations; sum via a log2 tree of
  2x-mode adds.
- rhs = [wfull * xl_raw | w]: the message is alpha * xl[src], so the PE
  scatter (S^T @ rhs accumulated over groups) directly yields
  sum_e w*xl*[dst==j] and the softmax denominators; epilogue is just
  out = relu(psum_feat / denom + bias') with bias' = s2*b2 folding.
- w broadcast and the scatter one-hot build use pair-duplicate +
  int32-view copies so the wide expansions run at 2x instead of 1x.
- Between layers: each core computes xl2~ = h1 @ W2l' per block in an
  interlude; a 4-chunk AllGather (bf16) replicates the xl2~ table with
  most chunks overlapping the layer-1 tail; xr2~ stays local.
"""
import sys
sys.path.insert(0, '/opt/trn_rl_repo')
import numpy as np
from dataclasses import dataclass, field

import concourse.bass as bass
import concourse.bacc as bacc
import concourse.mybir as mybir
from concourse.tile import TileContext
from concourse.library_config import mlp
from concourse.bass_utils import run_bass_kernel_spmd

P = 128
H, C = 4, 32
D = H * C          # 128
SLOPE = 0.2
F32 = mybir.dt.float32
BF16 = mybir.dt.bfloat16
I16 = mybir.dt.int16
I32 = mybir.dt.int32
NPBF = mybir.dt.np(BF16)
NCHUNK = 4         # AllGather chunks


@dataclass
class Plan:
    N: int
    NC: int
    NPC: int        # nodes per core
    NBLK: int       # blocks per core
    SLAB: int       # NBLK*128
    split_rank: int
    # per-layer, per-block group counts (lo/hi src-table sides)
    G1_lo: list = field(default_factory=list)
    G1_hi: list = field(default_factory=list)
    G2_lo: list = field(default_factory=list)
    G2_hi: list = field(default_factory=list)
    chunk_blocks: list = field(default_factory=list)  # AG chunk boundaries

    @property
    def GMAX(self):
        return max(max(a + b for a, b in zip(self.G1_lo, self.G1_hi)),
                   max(a + b for a, b in zip(self.G2_lo, self.G2_hi)))

    @property
    def GSIDE(self):
        return max(max(self.G1_lo), max(self.G1_hi),
                   max(self.G2_lo), max(self.G2_hi))


def wrap_idx(flat):
    """[n] int -> dma_gather SBUF layout [128, n//16] (16-wrapped, 8x replicated)."""
    n = flat.shape[0]
    assert n % 16 == 0
    w = flat.reshape(n // 16, 16).T      # [16, n/16]
    return np.tile(w, (8, 1)).astype(np.int16)


def chunk_layout(plan):
    """AllGather chunking: chunk c covers blocks [b0, b1). Returns
    (bounds list of (b0, b1), row base of chunk c in xl2_full, split chunk)."""
    NBLK = plan.NBLK
    per = NBLK // NCHUNK
    bounds = []
    b0 = 0
    for c in range(NCHUNK):
        b1 = b0 + per + (1 if c >= NCHUNK - (NBLK - per * NCHUNK) else 0)
        bounds.append((b0, b1))
        b0 = b1
    assert b0 == NBLK
    bases = []
    base = 0
    for (b0, b1) in bounds:
        bases.append(base)
        base += plan.NC * (b1 - b0) * P
    return bounds, bases


def l2_addr(plan, bounds, bases, g):
    """Global node id (core k, slab row r) -> row in chunked xl2_full."""
    k = g // plan.NPC
    r = g % plan.NPC
    b = r // P
    for c, (b0, b1) in enumerate(bounds):
        if b0 <= b < b1:
            return bases[c] + k * (b1 - b0) * P + (r - b0 * P)
    raise AssertionError


def preprocess(x, edge_index, NC=8):
    """Build the per-core streams. Returns (plan, per_core_dict_list)."""
    N = x.shape[0]
    assert N % NC == 0
    NPC = N // NC
    NBLK = (NPC + P - 1) // P
    SLAB = NBLK * P
    split_rank = NC // 2
    SPLIT1 = split_rank * NPC          # layer-1 lo/hi split (global node id)
    assert SPLIT1 <= 32768 and N - SPLIT1 <= 32768

    loop = np.arange(N, dtype=np.int64)
    src = np.concatenate([np.asarray(edge_index[0]), loop]).astype(np.int64)
    dst = np.concatenate([np.asarray(edge_index[1]), loop]).astype(np.int64)

    order = np.argsort(dst, kind='stable')
    src = src[order].astype(np.int32)
    dst = dst[order].astype(np.int32)

    core_bounds = np.searchsorted(dst, np.arange(NC + 1) * NPC)

    plan = Plan(N=N, NC=NC, NPC=NPC, NBLK=NBLK, SLAB=SLAB,
                split_rank=split_rank)
    bounds, bases = chunk_layout(plan)
    plan.chunk_blocks = bounds

    # layer-2 split: chunked xl2_full table rows, split at a chunk boundary
    # such that both halves stay < 32768
    split_chunk = None
    for c in range(1, NCHUNK):
        lo_rows = bases[c]
        if lo_rows <= 32768 and (NC * SLAB - lo_rows) <= 32768:
            split_chunk = c
    assert split_chunk is not None
    SPLIT2 = bases[split_chunk]

    # map global node id -> chunked xl2_full row
    def l2_rows(g):
        k = g // NPC
        r = g % NPC
        b = r // P
        row = np.zeros_like(g)
        for c, (b0, b1) in enumerate(bounds):
            m = (b >= b0) & (b < b1)
            row[m] = bases[c] + k[m] * (b1 - b0) * P + (r[m] - b0 * P)
        return row

    per_core = []
    cnts = np.zeros((4, NC, NBLK), np.int64)   # l1lo, l1hi, l2lo, l2hi
    for k in range(NC):
        a, b = core_bounds[k], core_bounds[k + 1]
        s_k = src[a:b]
        d_k = dst[a:b] - k * NPC
        blk = d_k // P
        r2 = l2_rows(s_k.astype(np.int64)).astype(np.int32)
        lo1 = s_k < SPLIT1
        lo2 = r2 < SPLIT2
        cnts[0, k] = np.bincount(blk[lo1], minlength=NBLK)
        cnts[1, k] = np.bincount(blk[~lo1], minlength=NBLK)
        cnts[2, k] = np.bincount(blk[lo2], minlength=NBLK)
        cnts[3, k] = np.bincount(blk[~lo2], minlength=NBLK)
        per_core.append((s_k, d_k, blk, r2, lo1, lo2))

    G = [[int(v) for v in np.max((cnts[i] + P - 1) // P, axis=0)]
         for i in range(4)]
    plan.G1_lo, plan.G1_hi, plan.G2_lo, plan.G2_hi = G
    GMAX = plan.GMAX
    W = 2 * GMAX * 8

    def _pack(G_lo, G_hi, blk, is_lo, d_k, aidx):
        """One layer's idx stream + dstrel. Pads use idx 0 (a valid row):
        num_idxs_reg must equal the non-negative idx count, which must be
        uniform across cores for one SPMD program; dstrel=-1 zeroes pads."""
        bi = np.zeros((NBLK, P, W), np.int16)
        dr = np.full((NBLK, GMAX * P), -1.0, np.float32)
        for b in range(NBLK):
            in_b = blk == b
            gl, gh = G_lo[b], G_hi[b]
            segsA, segsB = [], []
            for side, Gn in ((True, gl), (False, gh)):
                sel = in_b & (is_lo == side)
                ii = aidx[sel]
                dd = d_k[sel]
                n = ii.shape[0]
                assert n <= Gn * P
                a = np.zeros(Gn * P, np.int32)
                a[:n] = ii
                segsA.append(a)
                ab = np.zeros(Gn * P, np.int32)
                ab[:n] = dd
                segsB.append(ab)
                o = (0 if side else gl) * P
                drw = np.full(Gn * P, -1.0, np.float32)
                drw[:n] = dd - b * P
                dr[b, o:o + Gn * P] = drw
            col = 0
            for seg in segsA + segsB:
                wseg = wrap_idx(seg)
                bi[b, :, col:col + wseg.shape[1]] = wseg
                col += wseg.shape[1]
        drv = dr.reshape(NBLK, GMAX, P).transpose(0, 2, 1).reshape(
            NBLK * P, GMAX)
        return (bi.reshape(NBLK * P, W),
                np.ascontiguousarray(drv).astype(NPBF))

    datas = []
    for k in range(NC):
        s_k, d_k, blk, r2, lo1, lo2 = per_core[k]
        a1 = np.where(lo1, s_k, s_k - SPLIT1)
        a2 = np.where(lo2, r2, r2 - SPLIT2)
        bi1, dr1 = _pack(plan.G1_lo, plan.G1_hi, blk, lo1, d_k, a1)
        bi2, dr2 = _pack(plan.G2_lo, plan.G2_hi, blk, lo2, d_k, a2)
        datas.append(dict(blkidx_l1=bi1, blkidx_l2=bi2,
                          dstrel1=dr1, dstrel2=dr2))
    plan.SPLIT2 = SPLIT2
    plan.chunk_bases = bases
    return plan, datas


def sign_perm(att):
    """Permutation putting each head's positive-att columns first.

    Returns (perm[128], scales s=|att|[perm], pos-counts per head)."""
    a = np.asarray(att, np.float32).reshape(H, C)
    perm = []
    pcounts = []
    for h in range(H):
        pos = np.nonzero(a[h] >= 0)[0]
        neg = np.nonzero(a[h] < 0)[0]
        perm.extend((h * C + pos).tolist() + (h * C + neg).tolist())
        pcounts.append(len(pos))
    perm = np.asarray(perm, np.int64)
    flat = np.abs(a.reshape(-1))[perm]
    return perm, flat.astype(np.float32), pcounts


def build_kernel(plan, pcounts1, pcounts2, repeat=1, scratch=16384,
                 dr_act=False, pe_add=False, gch=8, inplace_act=False,
                 skew=True, s_early=True, qmode='single', fake_gather=0):
    """Build the SPMD nc program (identical for all cores)."""
    pl = plan
    NBLK, SLAB = pl.NBLK, pl.SLAB
    GMAX = pl.GMAX
    GSIDE = pl.GSIDE
    NLO1 = pl.split_rank * pl.NPC
    SPLIT2 = pl.SPLIT2
    bounds = pl.chunk_blocks
    bases = pl.chunk_bases
    A = mybir.AluOpType

    # ucode ring need per gather call: num_idxs/16 + 1 descs per DMA engine,
    # bounded by the carveout ring (scratch/16 descs)
    ring_cap = scratch // 16
    assert (min(gch, GSIDE) * P) // 16 + 1 <= ring_cap, (gch, GSIDE, ring_cap)

    # DMASW sem lanes (8) are assigned to SWDGE DMAs in SCHEDULED order, and
    # each lane is locked to one queue; program-order round-robin can violate
    # that when the scheduler reorders gathers. Safe modes:
    #  - 'single': one queue for everything.
    #  - 'parity': queue = block % 2. Requires exactly 4 gather calls per
    #    block (gch >= GSIDE, no empty sides): each block covers 4 of the 8
    #    lanes, so any within-block reorder keeps each lane on one queue.
    #  - dict: explicit per-gather queue map {emission index: queue} from
    #    build_queue_map's two-pass scheduled-order assignment.
    if qmode == 'parity':
        assert gch >= GSIDE
        assert min(min(pl.G1_lo), min(pl.G1_hi),
                   min(pl.G2_lo), min(pl.G2_hi)) >= 1
        NQ = 2
    elif isinstance(qmode, dict):
        NQ = 4
    else:
        NQ = 1
    nc = bacc.Bacc("TRN2", target_bir_lowering=False, debug=False,
                   num_swdge_queues=NQ, dynamic_dma_scratch_size=scratch)
    qctr = [0]
    gather_names = []
    dp = lambda name, shape, dt=BF16, out=False: nc.declare_dram_parameter(
        name, list(shape), dt, isOutput=out).ap()

    xl1 = dp("xl1", [pl.N, D])
    xr1_loc = dp("xr1_loc", [SLAB, D])
    blkidx_l1 = dp("blkidx_l1", [NBLK * P, 2 * GMAX * 8], I16)
    blkidx_l2 = dp("blkidx_l2", [NBLK * P, 2 * GMAX * 8], I16)
    dstrel1_p = dp("dstrel1", [NBLK * P, GMAX])
    dstrel2_p = dp("dstrel2", [NBLK * P, GMAX])
    iota_p = dp("iota", [P, P])
    ident_p = dp("ident", [P, P])
    W2l_p = dp("W2l", [D, D])
    W2r_p = dp("W2r", [D, D])
    bias1_p = dp("bias1", [P, D], F32)
    bias2_p = dp("bias2", [P, D], F32)
    out_p = dp("out", [SLAB, D], out=True)

    h1_loc = nc.dram_tensor("h1_loc", [SLAB, D], BF16).ap()
    xl2_slab = nc.dram_tensor("xl2_slab", [SLAB, D], BF16).ap()
    xl2_full = nc.dram_tensor("xl2_full", [pl.NC * SLAB, D], BF16,
                              addr_space="Shared").ap()
    xr2_loc = nc.dram_tensor("xr2_loc", [SLAB, D], BF16).ap()

    with TileContext(nc) as tc:
        nc.gpsimd.load_library(mlp)
        with (
            tc.tile_pool(name="const", bufs=1) as cpool,
            tc.tile_pool(name="stream", bufs=3) as spool,
            tc.tile_pool(name="gat", bufs=2) as gpool,
            tc.tile_pool(name="work", bufs=2) as wpool,
            tc.tile_pool(name="small", bufs=2) as smpool,
            tc.tile_pool(name="psum", bufs=2, space="PSUM") as pspool,
            tc.tile_pool(name="psum2", bufs=2, space="PSUM") as ps2pool,
            tc.tile_pool(name="psumz", bufs=2, space="PSUM") as pszpool,
        ):
            iota_c = cpool.tile([P, P], BF16)
            nc.sync.dma_start(out=iota_c[:], in_=iota_p[:, :])
            ident_c = cpool.tile([P, P], BF16)
            nc.sync.dma_start(out=ident_c[:], in_=ident_p[:, :])
            W2l_c = cpool.tile([D, D], BF16)
            nc.sync.dma_start(out=W2l_c[:], in_=W2l_p[:, :])
            W2r_c = cpool.tile([D, D], BF16)
            nc.sync.dma_start(out=W2r_c[:], in_=W2r_p[:, :])
            bias1_c = cpool.tile([P, D], F32)
            nc.sync.dma_start(out=bias1_c[:], in_=bias1_p[:, :])
            bias2_c = cpool.tile([P, D], F32)
            nc.sync.dma_start(out=bias2_c[:], in_=bias2_p[:, :])
            alpha_c = cpool.tile([P, 1], F32)
            nc.vector.memset(alpha_c[:], SLOPE)

            def front(b, G_lo, G_hi, tab_lo, tab_hi, tab_B, blkidx, dstrel_p,
                      pcounts):
                """Gathers + score pipeline + S build; returns state for
                back(). With s_early the S build is emitted right after the
                add so DVE fills the ACT-Prelu bubble."""
                gl, gh = G_lo[b], G_hi[b]
                gpb = gl + gh
                idx_t = spool.tile([P, 2 * GMAX * 8], I16, tag="idx")
                nc.sync.dma_start(out=idx_t[:, 0:2 * gpb * 8],
                                  in_=blkidx[b * P:(b + 1) * P, 0:2 * gpb * 8])
                dr_t = spool.tile([P, GMAX], BF16, tag="dr")
                nc.sync.dma_start(out=dr_t[:, 0:gpb],
                                  in_=dstrel_p[b * P:(b + 1) * P, 0:gpb])

                za = gpool.tile([P, GMAX, D], BF16, tag="za")
                zb = gpool.tile([P, GMAX, D], BF16, tag="zb")
                segs = [(0, gl, tab_lo), (gl, gh, tab_hi)]
                segs = [s for s in segs if s[1] > 0]
                for tgt, boff in ((za, 0), (zb, gpb)):
                    for G0, Gn, tab in segs:
                        t = tab_B if boff else tab
                        for g0 in range(0, Gn, gch):
                            gn = min(gch, Gn - g0)
                            if isinstance(qmode, dict):
                                qn = qmode.get(qctr[0], 0)
                            elif qmode == 'parity':
                                qn = b % 2
                            else:
                                qn = 0
                            # fake_gather bits: 1 = fake the A (src) stream,
                            # 2 = fake the B (dst) stream
                            if fake_gather & (2 if boff else 1):
                                # ablation: same bytes, contiguous HWDGE load
                                nc.sync.dma_start(
                                    out=tgt[:, G0 + g0:G0 + g0 + gn, :],
                                    in_=t[0:gn * P, :].rearrange(
                                        "(q p) d -> p q d", p=P))
                                continue
                            gi_inst = nc.gpsimd.dma_gather(
                                out_ap=tgt[:, G0 + g0:G0 + g0 + gn, :],
                                in_ap=t,
                                idxs_ap=idx_t[:, (boff + G0 + g0) * 8:
                                              (boff + G0 + g0 + gn) * 8],
                                num_idxs=gn * P, num_idxs_reg=gn * P,
                                elem_size=D, queue_num=qn)
                            gather_names.append(gi_inst.ins.name)
                            qctr[0] += 1

                def s_build():
                    # S = (iota == dstrel) via pair-dup + int32-view expand
                    drfull = wpool.tile([P, GMAX, P], BF16, tag="drfull")
                    if dr_act:
                        nc.scalar.activation(
                            out=drfull[:, 0:gpb],
                            in_=dr_t[:, 0:gpb].unsqueeze(2).to_broadcast(
                                [P, gpb, P]),
                            func=mybir.ActivationFunctionType.Copy)
                    else:
                        drdup = smpool.tile([P, GMAX, 2], BF16, tag="drdup")
                        nc.vector.tensor_copy(
                            out=drdup[:, 0:gpb],
                            in_=dr_t[:, 0:gpb].unsqueeze(2).to_broadcast(
                                [P, gpb, 2]))
                        nc.vector.tensor_copy(
                            out=drfull[:, 0:gpb].bitcast(I32),
                            in_=drdup[:, 0:gpb].bitcast(I32).to_broadcast(
                                [P, gpb, P // 2]))
                    S_t = wpool.tile([P, GMAX, P], BF16, tag="S")
                    nc.vector.tensor_tensor(
                        out=S_t[:, 0:gpb],
                        in0=iota_c[:].unsqueeze(1).to_broadcast([P, gpb, P]),
                        in1=drfull[:, 0:gpb], op=A.is_equal)
                    return S_t

                # z~ = xl~ + xr~ (in-place into zb; raw xr~ is dead after the
                # add: the message side uses raw za only). Prelu lands in zw,
                # which is later overwritten by the wfull expansion (the tree
                # has consumed it by then).
                zw = wpool.tile([P, GMAX, D], BF16, tag="wfull")
                if pe_add:
                    # PE identity-matmul add into f32 PSUM chunks; ACT Prelu
                    # drains each chunk into zw
                    CH = 512
                    zaf = za[:, 0:gpb].rearrange("p g d -> p (g d)")
                    zbf = zb[:, 0:gpb].rearrange("p g d -> p (g d)")
                    zwf = zw[:, 0:gpb].rearrange("p g d -> p (g d)")
                    tot = gpb * D
                    for c0 in range(0, tot, CH):
                        w = min(CH, tot - c0)
                        zps = pszpool.tile([P, CH], F32, tag="zadd")
                        nc.tensor.matmul(out=zps[:, 0:w], lhsT=ident_c[:],
                                         rhs=zaf[:, c0:c0 + w],
                                         start=True, stop=False)
                        nc.tensor.matmul(out=zps[:, 0:w], lhsT=ident_c[:],
                                         rhs=zbf[:, c0:c0 + w],
                                         start=False, stop=True)
                        nc.scalar.activation(
                            out=zwf[:, c0:c0 + w], in_=zps[:, 0:w],
                            func=mybir.ActivationFunctionType.Prelu,
                            alpha=alpha_c[:, :])
                else:
                    nc.vector.tensor_tensor(out=zb[:, 0:gpb],
                                            in0=za[:, 0:gpb],
                                            in1=zb[:, 0:gpb], op=A.add)
                    nc.scalar.activation(
                        out=(zb if inplace_act else zw)[:, 0:gpb],
                        in_=zb[:, 0:gpb],
                        func=mybir.ActivationFunctionType.Prelu,
                        alpha=alpha_c[:, :])
                    if inplace_act:
                        zw = zb
                # S build emitted here fills the DVE bubble while the ACT
                # engine runs the Prelu
                S_t = s_build() if s_early else None
                # sign fix: negate each head's negative-att column block
                zbh = zw[:].rearrange("p g (h c) -> p g h c", h=H)
                for h in range(H):
                    ph = pcounts[h]
                    if ph < C:
                        nc.vector.tensor_scalar_mul(
                            out=zbh[:, 0:gpb, h, ph:C],
                            in0=zbh[:, 0:gpb, h, ph:C], scalar1=-1.0)
                # tree-reduce over c (2x-mode adds; last step to fp32)
                e16 = smpool.tile([P, GMAX, H, 16], BF16, tag="e16")
                nc.vector.tensor_tensor(out=e16[:, 0:gpb],
                                        in0=zbh[:, 0:gpb, :, 0:16],
                                        in1=zbh[:, 0:gpb, :, 16:32], op=A.add)
                e8 = smpool.tile([P, GMAX, H, 8], BF16, tag="e8")
                nc.vector.tensor_tensor(out=e8[:, 0:gpb],
                                        in0=e16[:, 0:gpb, :, 0:8],
                                        in1=e16[:, 0:gpb, :, 8:16], op=A.add)
                e4 = smpool.tile([P, GMAX, H, 4], BF16, tag="e4")
                nc.vector.tensor_tensor(out=e4[:, 0:gpb],
                                        in0=e8[:, 0:gpb, :, 0:4],
                                        in1=e8[:, 0:gpb, :, 4:8], op=A.add)
                e2 = smpool.tile([P, GMAX, H, 2], BF16, tag="e2")
                nc.vector.tensor_tensor(out=e2[:, 0:gpb],
                                        in0=e4[:, 0:gpb, :, 0:2],
                                        in1=e4[:, 0:gpb, :, 2:4], op=A.add)
                e1 = smpool.tile([P, GMAX, H], F32, tag="e1")
                nc.vector.tensor_tensor(out=e1[:, 0:gpb],
                                        in0=e2[:, 0:gpb, :, 0],
                                        in1=e2[:, 0:gpb, :, 1], op=A.add)
                # w = exp(e)
                w_t = smpool.tile([P, GMAX, H], BF16, tag="w")
                nc.scalar.activation(out=w_t[:, 0:gpb], in_=e1[:, 0:gpb],
                                     func=mybir.ActivationFunctionType.Exp)
                # expand w to [P,G,H,C] via pair-dup + int32-view copy
                wdup = smpool.tile([P, GMAX, H, 2], BF16, tag="wdup")
                nc.vector.tensor_copy(
                    out=wdup[:, 0:gpb],
                    in_=w_t[:, 0:gpb].unsqueeze(3).to_broadcast(
                        [P, gpb, H, 2]))
                wfull = zw if not inplace_act else wpool.tile(
                    [P, GMAX, D], BF16, tag="wf2")
                nc.vector.tensor_copy(
                    out=wfull[:, 0:gpb].bitcast(I32).rearrange(
                        "p g (h c) -> p g h c", h=H),
                    in_=wdup[:, 0:gpb].bitcast(I32).to_broadcast(
                        [P, gpb, H, C // 2]))
                # rhs = [wfull * xl_raw | w]
                rhs = wpool.tile([P, GMAX, D + H], BF16, tag="rhs")
                nc.vector.tensor_tensor(out=rhs[:, 0:gpb, 0:D],
                                        in0=za[:, 0:gpb], in1=wfull[:, 0:gpb],
                                        op=A.mult)
                nc.vector.tensor_copy(out=rhs[:, 0:gpb, D:D + H],
                                      in_=w_t[:, 0:gpb])
                if not s_early:
                    S_t = s_build()
                return (b, gpb, S_t, rhs)

            def back(st, bias_c, out_rows, interlude=None):
                b, gpb, S_t, rhs = st
                ps = pspool.tile([P, D + H], F32, tag="agg")
                for gi in range(gpb):
                    nc.tensor.matmul(
                        out=ps[:], lhsT=S_t[:, gi, :], rhs=rhs[:, gi, :],
                        start=(gi == 0), stop=(gi == gpb - 1))

                # denom > 0 always: every node has a self-loop edge
                dinv = smpool.tile([P, H], F32, tag="dinv")
                nc.vector.reciprocal(out=dinv[:], in_=ps[:, D:D + H])
                t3 = smpool.tile([P, D], F32, tag="t3")
                nc.vector.tensor_tensor(
                    out=t3[:].rearrange("p (h c) -> p h c", h=H),
                    in0=ps[:, 0:D].rearrange("p (h c) -> p h c", h=H),
                    in1=dinv[:].unsqueeze(2).to_broadcast([P, H, C]),
                    op=A.mult)
                t4 = smpool.tile([P, D], F32, tag="t4")
                nc.vector.tensor_tensor(out=t4[:], in0=t3[:], in1=bias_c[:],
                                        op=A.add)
                hrow = smpool.tile([P, D], BF16, tag="hrow")
                nc.vector.tensor_scalar_max(out=hrow[:], in0=t4[:],
                                            scalar1=0.0)
                nc.sync.dma_start(out=out_rows[b * P:(b + 1) * P, :],
                                  in_=hrow[:])
                if interlude is not None:
                    interlude(b)

            def interlude(b):
                htile = smpool.tile([P, D], BF16, tag="pl_h")
                nc.sync.dma_start(out=htile[:],
                                  in_=h1_loc[b * P:(b + 1) * P, :])
                psT = ps2pool.tile([P, P], BF16, tag="pl_T")
                nc.tensor.transpose(out=psT[:], in_=htile[:],
                                    identity=ident_c[:])
                hT = smpool.tile([P, P], BF16, tag="pl_hT")
                nc.vector.tensor_copy(out=hT[:], in_=psT[:])
                for W_c, table in ((W2l_c, xl2_slab), (W2r_c, xr2_loc)):
                    psm = ps2pool.tile([P, D], F32, tag="pl_mm")
                    nc.tensor.matmul(out=psm[:], lhsT=hT[:], rhs=W_c[:],
                                     start=True, stop=True)
                    res = smpool.tile([P, D], BF16, tag="pl_res")
                    nc.vector.tensor_copy(out=res[:], in_=psm[:])
                    nc.sync.dma_start(out=table[b * P:(b + 1) * P, :],
                                      in_=res[:])
                # chunked AllGather: fire chunk c once its last block's
                # interlude has written xl2_slab rows
                for c, (b0, b1) in enumerate(bounds):
                    if b == b1 - 1:
                        nc.gpsimd.collective_compute(
                            "AllGather", A.bypass,
                            replica_groups=[list(range(pl.NC))],
                            ins=[xl2_slab[b0 * P:b1 * P, :].opt()],
                            outs=[xl2_full[bases[c]:
                                           bases[c] + pl.NC * (b1 - b0) * P,
                                           :].opt()],
                        )

            def layer(G_lo, G_hi, tab_lo, tab_hi, tab_B, blkidx, dstrel_p,
                      pcounts, bias_c, out_rows, inter):
                pend = None
                for b in range(NBLK):
                    st = front(b, G_lo, G_hi, tab_lo, tab_hi, tab_B, blkidx,
                               dstrel_p, pcounts)
                    if not skew:
                        back(st, bias_c, out_rows, inter)
                        continue
                    if pend is not None:
                        back(pend, bias_c, out_rows, inter)
                    pend = st
                if pend is not None:
                    back(pend, bias_c, out_rows, inter)

            for _rep in range(repeat):
                layer(pl.G1_lo, pl.G1_hi, xl1[0:NLO1, :], xl1[NLO1:pl.N, :],
                      xr1_loc[:, :], blkidx_l1, dstrel1_p, pcounts1, bias1_c,
                      h1_loc, interlude)
                layer(pl.G2_lo, pl.G2_hi, xl2_full[0:SPLIT2, :],
                      xl2_full[SPLIT2:pl.NC * SLAB, :], xr2_loc[:, :],
                      blkidx_l2, dstrel2_p, pcounts2, bias2_c, out_p, None)
    nc._gather_names = gather_names
    return nc


def _scheduled_swdge_order(nc):
    """Pool-engine DMA instruction names in SCHEDULED order (= DMASW sem
    lane assignment order)."""
    import concourse.bass_isa as bass_isa
    out = []
    for blk in nc.m.functions[0].blocks:
        for inst in blk.instructions:
            if (getattr(inst, 'engine', None) == mybir.EngineType.Pool
                    and isinstance(inst, bass_isa.AnyDMAInstruction)):
                out.append(inst.name)
    return out


def build_kernel_auto(plan, pcounts1, pcounts2, repeat=1, tries=4, **kw):
    """Two-pass queue assignment: compile, read the scheduled SWDGE order,
    rebuild with queue = scheduled-position % 4 so each of the 8 DMASW sem
    lanes (assigned in scheduled order) sees exactly one queue. Iterate in
    case the queue change perturbs the schedule; fall back to single queue."""
    kw.pop('qmode', None)
    qmode = 'single'
    nc_single = None
    for t in range(tries):
        nc = build_kernel(plan, pcounts1, pcounts2, repeat=repeat,
                          qmode=qmode, **kw)
        nc.compile()
        order = _scheduled_swdge_order(nc)
        pos = {n: i for i, n in enumerate(order)}
        em = nc._gather_names
        lanes = {}
        ok = True
        for ei, n in enumerate(em):
            lane = pos[n] % 8
            q = qmode.get(ei, 0) if isinstance(qmode, dict) else 0
            if lanes.setdefault(lane, q) != q:
                ok = False
        if ok and isinstance(qmode, dict):
            return nc
        if not isinstance(qmode, dict):
            nc_single = nc
        qmode = {ei: pos[n] % 4 for ei, n in enumerate(em)}
    if nc_single is None:
        nc_single = build_kernel(plan, pcounts1, pcounts2, repeat=repeat,
                                 qmode='single', **kw)
        nc_single.compile()
    return nc_single


def make_host_tables(x, W1_l, W1_r, att1, b1, W2_l, W2_r, att2, b2):
    """Host-side scaled/permuted tables and constants."""
    perm1, s1, pc1 = sign_perm(att1)
    perm2, s2, pc2 = sign_perm(att2)
    x = np.asarray(x, np.float32)
    xl1 = (x @ np.asarray(W1_l, np.float32))[:, perm1] * s1
    xr1 = (x @ np.asarray(W1_r, np.float32))[:, perm1] * s1
    # W2': rows in perm1 order, unscaled by 1/s1; cols in perm2 order, scaled
    W2l = (np.asarray(W2_l, np.float32)[perm1][:, perm2] * s2) / s1[:, None]
    W2r = (np.asarray(W2_r, np.float32)[perm1][:, perm2] * s2) / s1[:, None]
    bias1 = np.asarray(b1, np.float32)[perm1] * s1
    bias2 = np.asarray(b2, np.float32)[perm2] * s2
    return dict(perm1=perm1, s1=s1, pc1=pc1, perm2=perm2, s2=s2, pc2=pc2,
                xl1=xl1.astype(NPBF), xr1=xr1.astype(NPBF),
                W2l=W2l.astype(NPBF), W2r=W2r.astype(NPBF),
                bias1=np.tile(bias1[None, :], (P, 1)).astype(np.float32),
                bias2=np.tile(bias2[None, :], (P, 1)).astype(np.float32))


def make_inputs(plan, datas, ht):
    pl = plan
    iota = np.tile(np.arange(P, dtype=np.float32)[None, :], (P, 1)).astype(NPBF)
    ident = np.eye(P, dtype=np.float32).astype(NPBF)

    in_maps = []
    for k in range(pl.NC):
        xr1_loc = np.zeros((pl.SLAB, D), NPBF)
        nreal = min(pl.NPC, pl.N - k * pl.NPC)
        xr1_loc[:nreal] = ht['xr1'][k * pl.NPC: k * pl.NPC + nreal]
        in_maps.append(dict(
            xl1=ht['xl1'],
            xr1_loc=xr1_loc,
            blkidx_l1=datas[k]["blkidx_l1"],
            blkidx_l2=datas[k]["blkidx_l2"],
            dstrel1=datas[k]["dstrel1"],
            dstrel2=datas[k]["dstrel2"],
            iota=iota, ident=ident,
            W2l=ht['W2l'], W2r=ht['W2r'],
            bias1=ht['bias1'], bias2=ht['bias2'],
        ))
    return in_maps


def assemble_output(plan, results, ht):
    out = np.zeros((plan.N, D), np.float32)
    for k in range(plan.NC):
        out[k * plan.NPC:(k + 1) * plan.NPC] = \
            results[k]["out"][:plan.NPC].astype(np.float32)
    # undo layer-2 column scale+permutation
    full = np.empty_like(out)
    full[:, ht['perm2']] = out / ht['s2'][None, :]
    return full


def kernel(x, edge_index, W1_l, W1_r, att1, b1, W2_l, W2_r, att2, b2):
    x = np.ascontiguousarray(np.asarray(x, np.float32))
    edge_index = np.asarray(edge_index)
    plan, datas = preprocess(x, edge_index, NC=8)
    ht = make_host_tables(x, W1_l, W1_r, att1, b1, W2_l, W2_r, att2, b2)
    nc = build_kernel_auto(plan, ht['pc1'], ht['pc2'])
    in_maps = make_inputs(plan, datas, ht)
    res = run_bass_kernel_spmd(nc, in_maps, core_ids=list(range(8)))
    return assemble_output(plan, res.results, ht)


# revision 44
# speedup vs baseline: 1.0218x; 1.0218x over previous
"""GATv2 2-layer GNN message-passing kernel for Trainium2, 8-core SPMD.

Contract: kernel(**inputs) takes the FULL unsharded inputs (as produced by
setup_inputs) and returns the FULL [50000, 128] float32 output.

Strategy (edge/data parallel, dst-range sharded), v3:
- Host: append self-loops, sort edges by dst, give each of the 8 cores an
  equal contiguous dst-node range (6250 nodes = 49 blocks of 128). Within
  each block, edges are split by src-half so the int16 dma_gather indices
  stay < 32768 (two source tables); layer 2 gets its own split/slots since
  the chunked AllGather layout is not monotone in node id. Per-block group
  counts are padded to the max over the 8 cores (not the global max) so one
  SPMD program serves all cores with minimal slot waste.
- Fused per-block pipeline (no DRAM staging): per block, dma_gather calls
  (xl from the src tables, xr from the local dst table, <=1024 idx each)
  land in double-buffered SBUF tiles; compute consumes them directly.
- SWDGE queue assignment is the single biggest HW lever (measured 8.3ms ->
  3.5ms): the 8 DMASW sem lanes are given to SWDGE DMAs in SCHEDULED order
  and each lane is locked to one queue, so build_kernel_auto compiles once,
  reads the scheduled gather order, rebuilds with queue = position % 4, and
  re-verifies the lane->queue map (falling back to a single queue).
- All edge-path tensors are bf16. The gather tables are pre-scaled by
  |att| with columns permuted so each head's positive-att columns come
  first: the per-edge attention dot then reduces to
  e = sum_pos Prelu(z~) - sum_neg Prelu(z~), killing the att multiply.
  Sign fix via 4x-mode tensor_scalar negations; sum via a log2 tree of
  2x-mode adds.
- rhs = [wfull * xl_raw | w]: the message is alpha * xl[src], so the PE
  scatter (S^T @ rhs accumulated over groups) directly yields
  sum_e w*xl*[dst==j] and the softmax denominators; epilogue is just
  out = relu(psum_feat / denom + bias') with bias' = s2*b2 folding.
- w broadcast and the scatter one-hot build use pair-duplicate +
  int32-view copies so the wide expansions run at 2x instead of 1x.
- Between layers: each core computes xl2~ = h1 @ W2l' per block in an
  interlude; a 4-chunk AllGather (bf16) replicates the xl2~ table with
  most chunks overlapping the layer-1 tail; xr2~ stays local.
"""
import sys
sys.path.insert(0, '/opt/trn_rl_repo')
import numpy as np
from dataclasses import dataclass, field

import concourse.bass as bass
import concourse.bacc as bacc
import concourse.mybir as mybir
from concourse.tile import TileContext
from concourse.library_config import mlp
from concourse.bass_utils import run_bass_kernel_spmd

P = 128
H, C = 4, 32
D = H * C          # 128
SLOPE = 0.2
F32 = mybir.dt.float32
BF16 = mybir.dt.bfloat16
I16 = mybir.dt.int16
I32 = mybir.dt.int32
NPBF = mybir.dt.np(BF16)
NCHUNK = 4         # AllGather chunks


@dataclass
class Plan:
    N: int
    NC: int
    NPC: int        # nodes per core
    NBLK: int       # blocks per core
    SLAB: int       # NBLK*128
    split_rank: int
    # per-layer, per-block group counts (lo/hi src-table sides)
    G1_lo: list = field(default_factory=list)
    G1_hi: list = field(default_factory=list)
    G2_lo: list = field(default_factory=list)
    G2_hi: list = field(default_factory=list)
    chunk_blocks: list = field(default_factory=list)  # AG chunk boundaries

    @property
    def GMAX(self):
        return max(max(a + b for a, b in zip(self.G1_lo, self.G1_hi)),
                   max(a + b for a, b in zip(self.G2_lo, self.G2_hi)))

    @property
    def GSIDE(self):
        return max(max(self.G1_lo), max(self.G1_hi),
                   max(self.G2_lo), max(self.G2_hi))


def wrap_idx(flat):
    """[n] int -> dma_gather SBUF layout [128, n//16] (16-wrapped, 8x replicated)."""
    n = flat.shape[0]
    assert n % 16 == 0
    w = flat.reshape(n // 16, 16).T      # [16, n/16]
    return np.tile(w, (8, 1)).astype(np.int16)


def chunk_layout(plan):
    """AllGather chunking: chunk c covers blocks [b0, b1). Returns
    (bounds list of (b0, b1), row base of chunk c in xl2_full, split chunk)."""
    NBLK = plan.NBLK
    per = NBLK // NCHUNK
    bounds = []
    b0 = 0
    for c in range(NCHUNK):
        b1 = b0 + per + (1 if c >= NCHUNK - (NBLK - per * NCHUNK) else 0)
        bounds.append((b0, b1))
        b0 = b1
    assert b0 == NBLK
    bases = []
    base = 0
    for (b0, b1) in bounds:
        bases.append(base)
        base += plan.NC * (b1 - b0) * P
    return bounds, bases


def l2_addr(plan, bounds, bases, g):
    """Global node id (core k, slab row r) -> row in chunked xl2_full."""
    k = g // plan.NPC
    r = g % plan.NPC
    b = r // P
    for c, (b0, b1) in enumerate(bounds):
        if b0 <= b < b1:
            return bases[c] + k * (b1 - b0) * P + (r - b0 * P)
    raise AssertionError


def preprocess(x, edge_index, NC=8):
    """Build the per-core streams. Returns (plan, per_core_dict_list)."""
    N = x.shape[0]
    assert N % NC == 0
    NPC = N // NC
    NBLK = (NPC + P - 1) // P
    SLAB = NBLK * P
    split_rank = NC // 2
    SPLIT1 = split_rank * NPC          # layer-1 lo/hi split (global node id)
    assert SPLIT1 <= 32768 and N - SPLIT1 <= 32768

    loop = np.arange(N, dtype=np.int64)
    src = np.concatenate([np.asarray(edge_index[0]), loop]).astype(np.int64)
    dst = np.concatenate([np.asarray(edge_index[1]), loop]).astype(np.int64)

    order = np.argsort(dst, kind='stable')
    src = src[order].astype(np.int32)
    dst = dst[order].astype(np.int32)

    core_bounds = np.searchsorted(dst, np.arange(NC + 1) * NPC)

    plan = Plan(N=N, NC=NC, NPC=NPC, NBLK=NBLK, SLAB=SLAB,
                split_rank=split_rank)
    bounds, bases = chunk_layout(plan)
    plan.chunk_blocks = bounds

    # layer-2 split: chunked xl2_full table rows, split at a chunk boundary
    # such that both halves stay < 32768
    split_chunk = None
    for c in range(1, NCHUNK):
        lo_rows = bases[c]
        if lo_rows <= 32768 and (NC * SLAB - lo_rows) <= 32768:
            split_chunk = c
    assert split_chunk is not None
    SPLIT2 = bases[split_chunk]

    # map global node id -> chunked xl2_full row
    def l2_rows(g):
        k = g // NPC
        r = g % NPC
        b = r // P
        row = np.zeros_like(g)
        for c, (b0, b1) in enumerate(bounds):
            m = (b >= b0) & (b < b1)
            row[m] = bases[c] + k[m] * (b1 - b0) * P + (r[m] - b0 * P)
        return row

    per_core = []
    cnts = np.zeros((4, NC, NBLK), np.int64)   # l1lo, l1hi, l2lo, l2hi
    for k in range(NC):
        a, b = core_bounds[k], core_bounds[k + 1]
        s_k = src[a:b]
        d_k = dst[a:b] - k * NPC
        blk = d_k // P
        r2 = l2_rows(s_k.astype(np.int64)).astype(np.int32)
        lo1 = s_k < SPLIT1
        lo2 = r2 < SPLIT2
        cnts[0, k] = np.bincount(blk[lo1], minlength=NBLK)
        cnts[1, k] = np.bincount(blk[~lo1], minlength=NBLK)
        cnts[2, k] = np.bincount(blk[lo2], minlength=NBLK)
        cnts[3, k] = np.bincount(blk[~lo2], minlength=NBLK)
        per_core.append((s_k, d_k, blk, r2, lo1, lo2))

    G = [[int(v) for v in np.max((cnts[i] + P - 1) // P, axis=0)]
         for i in range(4)]
    plan.G1_lo, plan.G1_hi, plan.G2_lo, plan.G2_hi = G
    GMAX = plan.GMAX
    W = 2 * GMAX * 8

    def _pack(G_lo, G_hi, blk, is_lo, d_k, aidx):
        """One layer's idx stream + dstrel. Pads use idx 0 (a valid row):
        num_idxs_reg must equal the non-negative idx count, which must be
        uniform across cores for one SPMD program; dstrel=-1 zeroes pads."""
        bi = np.zeros((NBLK, P, W), np.int16)
        dr = np.full((NBLK, GMAX * P), -1.0, np.float32)
        for b in range(NBLK):
            in_b = blk == b
            gl, gh = G_lo[b], G_hi[b]
            segsA, segsB = [], []
            for side, Gn in ((True, gl), (False, gh)):
                sel = in_b & (is_lo == side)
                ii = aidx[sel]
                dd = d_k[sel]
                n = ii.shape[0]
                assert n <= Gn * P
                a = np.zeros(Gn * P, np.int32)
                a[:n] = ii
                segsA.append(a)
                ab = np.zeros(Gn * P, np.int32)
                ab[:n] = dd
                segsB.append(ab)
                o = (0 if side else gl) * P
                drw = np.full(Gn * P, -1.0, np.float32)
                drw[:n] = dd - b * P
                dr[b, o:o + Gn * P] = drw
            col = 0
            for seg in segsA + segsB:
                wseg = wrap_idx(seg)
                bi[b, :, col:col + wseg.shape[1]] = wseg
                col += wseg.shape[1]
        drv = dr.reshape(NBLK, GMAX, P).transpose(0, 2, 1).reshape(
            NBLK * P, GMAX)
        return (bi.reshape(NBLK * P, W),
                np.ascontiguousarray(drv).astype(NPBF))

    datas = []
    for k in range(NC):
        s_k, d_k, blk, r2, lo1, lo2 = per_core[k]
        a1 = np.where(lo1, s_k, s_k - SPLIT1)
        a2 = np.where(lo2, r2, r2 - SPLIT2)
        bi1, dr1 = _pack(plan.G1_lo, plan.G1_hi, blk, lo1, d_k, a1)
        bi2, dr2 = _pack(plan.G2_lo, plan.G2_hi, blk, lo2, d_k, a2)
        datas.append(dict(blkidx_l1=bi1, blkidx_l2=bi2,
                          dstrel1=dr1, dstrel2=dr2))
    plan.SPLIT2 = SPLIT2
    plan.chunk_bases = bases
    return plan, datas


def sign_perm(att):
    """Permutation putting each head's positive-att columns first.

    Returns (perm[128], scales s=|att|[perm], pos-counts per head)."""
    a = np.asarray(att, np.float32).reshape(H, C)
    perm = []
    pcounts = []
    for h in range(H):
        pos = np.nonzero(a[h] >= 0)[0]
        neg = np.nonzero(a[h] < 0)[0]
        perm.extend((h * C + pos).tolist() + (h * C + neg).tolist())
        pcounts.append(len(pos))
    perm = np.asarray(perm, np.int64)
    flat = np.abs(a.reshape(-1))[perm]
    return perm, flat.astype(np.float32), pcounts


def build_kernel(plan, pcounts1, pcounts2, repeat=1, scratch=16384,
                 dr_act=False, pe_add=False, gch=8, inplace_act=False,
                 skew=True, s_early=True, qmode='single', fake_gather=0,
                 zbufs=2):
    """Build the SPMD nc program (identical for all cores)."""
    pl = plan
    NBLK, SLAB = pl.NBLK, pl.SLAB
    GMAX = pl.GMAX
    GSIDE = pl.GSIDE
    NLO1 = pl.split_rank * pl.NPC
    SPLIT2 = pl.SPLIT2
    bounds = pl.chunk_blocks
    bases = pl.chunk_bases
    A = mybir.AluOpType

    # ucode ring need per gather call: num_idxs/16 + 1 descs per DMA engine,
    # bounded by the carveout ring (scratch/16 descs)
    ring_cap = scratch // 16
    assert (min(gch, GSIDE) * P) // 16 + 1 <= ring_cap, (gch, GSIDE, ring_cap)

    # DMASW sem lanes (8) are assigned to SWDGE DMAs in SCHEDULED order, and
    # each lane is locked to one queue; program-order round-robin can violate
    # that when the scheduler reorders gathers. Safe modes:
    #  - 'single': one queue for everything.
    #  - 'parity': queue = block % 2. Requires exactly 4 gather calls per
    #    block (gch >= GSIDE, no empty sides): each block covers 4 of the 8
    #    lanes, so any within-block reorder keeps each lane on one queue.
    #  - dict: explicit per-gather queue map {emission index: queue} from
    #    build_queue_map's two-pass scheduled-order assignment.
    if qmode == 'parity':
        assert gch >= GSIDE
        assert min(min(pl.G1_lo), min(pl.G1_hi),
                   min(pl.G2_lo), min(pl.G2_hi)) >= 1
        NQ = 2
    elif isinstance(qmode, dict):
        NQ = 4
    else:
        NQ = 1
    nc = bacc.Bacc("TRN2", target_bir_lowering=False, debug=False,
                   num_swdge_queues=NQ, dynamic_dma_scratch_size=scratch)
    qctr = [0]
    gather_names = []
    dp = lambda name, shape, dt=BF16, out=False: nc.declare_dram_parameter(
        name, list(shape), dt, isOutput=out).ap()

    xl1 = dp("xl1", [pl.N, D])
    xr1_loc = dp("xr1_loc", [SLAB, D])
    blkidx_l1 = dp("blkidx_l1", [NBLK * P, 2 * GMAX * 8], I16)
    blkidx_l2 = dp("blkidx_l2", [NBLK * P, 2 * GMAX * 8], I16)
    dstrel1_p = dp("dstrel1", [NBLK * P, GMAX])
    dstrel2_p = dp("dstrel2", [NBLK * P, GMAX])
    iota_p = dp("iota", [P, P])
    ident_p = dp("ident", [P, P])
    W2l_p = dp("W2l", [D, D])
    W2r_p = dp("W2r", [D, D])
    bias1_p = dp("bias1", [P, D], F32)
    bias2_p = dp("bias2", [P, D], F32)
    out_p = dp("out", [SLAB, D], out=True)

    h1_loc = nc.dram_tensor("h1_loc", [SLAB, D], BF16).ap()
    xl2_slab = nc.dram_tensor("xl2_slab", [SLAB, D], BF16).ap()
    xl2_full = nc.dram_tensor("xl2_full", [pl.NC * SLAB, D], BF16,
                              addr_space="Shared").ap()
    xr2_loc = nc.dram_tensor("xr2_loc", [SLAB, D], BF16).ap()

    with TileContext(nc) as tc:
        nc.gpsimd.load_library(mlp)
        with (
            tc.tile_pool(name="const", bufs=1) as cpool,
            tc.tile_pool(name="stream", bufs=zbufs + 1) as spool,
            tc.tile_pool(name="gat", bufs=zbufs) as gpool,
            tc.tile_pool(name="work", bufs=2) as wpool,
            tc.tile_pool(name="small", bufs=2) as smpool,
            tc.tile_pool(name="psum", bufs=2, space="PSUM") as pspool,
            tc.tile_pool(name="psum2", bufs=2, space="PSUM") as ps2pool,
            tc.tile_pool(name="psumz", bufs=2, space="PSUM") as pszpool,
        ):
            iota_c = cpool.tile([P, P], BF16)
            nc.sync.dma_start(out=iota_c[:], in_=iota_p[:, :])
            ident_c = cpool.tile([P, P], BF16)
            nc.sync.dma_start(out=ident_c[:], in_=ident_p[:, :])
            W2l_c = cpool.tile([D, D], BF16)
            nc.sync.dma_start(out=W2l_c[:], in_=W2l_p[:, :])
            W2r_c = cpool.tile([D, D], BF16)
            nc.sync.dma_start(out=W2r_c[:], in_=W2r_p[:, :])
            bias1_c = cpool.tile([P, D], F32)
            nc.sync.dma_start(out=bias1_c[:], in_=bias1_p[:, :])
            bias2_c = cpool.tile([P, D], F32)
            nc.sync.dma_start(out=bias2_c[:], in_=bias2_p[:, :])
            alpha_c = cpool.tile([P, 1], F32)
            nc.vector.memset(alpha_c[:], SLOPE)

            def front(b, G_lo, G_hi, tab_lo, tab_hi, tab_B, blkidx, dstrel_p,
                      pcounts):
                """Gathers + score pipeline + S build; returns state for
                back(). With s_early the S build is emitted right after the
                add so DVE fills the ACT-Prelu bubble."""
                gl, gh = G_lo[b], G_hi[b]
                gpb = gl + gh
                idx_t = spool.tile([P, 2 * GMAX * 8], I16, tag="idx")
                nc.sync.dma_start(out=idx_t[:, 0:2 * gpb * 8],
                                  in_=blkidx[b * P:(b + 1) * P, 0:2 * gpb * 8])
                dr_t = spool.tile([P, GMAX], BF16, tag="dr")
                nc.sync.dma_start(out=dr_t[:, 0:gpb],
                                  in_=dstrel_p[b * P:(b + 1) * P, 0:gpb])

                za = gpool.tile([P, GMAX, D], BF16, tag="za")
                zb = gpool.tile([P, GMAX, D], BF16, tag="zb")
                segs = [(0, gl, tab_lo), (gl, gh, tab_hi)]
                segs = [s for s in segs if s[1] > 0]
                for tgt, boff in ((za, 0), (zb, gpb)):
                    for G0, Gn, tab in segs:
                        t = tab_B if boff else tab
                        for g0 in range(0, Gn, gch):
                            gn = min(gch, Gn - g0)
                            if isinstance(qmode, dict):
                                qn = qmode.get(qctr[0], 0)
                            elif qmode == 'parity':
                                qn = b % 2
                            else:
                                qn = 0
                            # fake_gather bits: 1 = fake the A (src) stream,
                            # 2 = fake the B (dst) stream
                            if fake_gather & (2 if boff else 1):
                                # ablation: same bytes, contiguous HWDGE load
                                nc.sync.dma_start(
                                    out=tgt[:, G0 + g0:G0 + g0 + gn, :],
                                    in_=t[0:gn * P, :].rearrange(
                                        "(q p) d -> p q d", p=P))
                                continue
                            gi_inst = nc.gpsimd.dma_gather(
                                out_ap=tgt[:, G0 + g0:G0 + g0 + gn, :],
                                in_ap=t,
                                idxs_ap=idx_t[:, (boff + G0 + g0) * 8:
                                              (boff + G0 + g0 + gn) * 8],
                                num_idxs=gn * P, num_idxs_reg=gn * P,
                                elem_size=D, queue_num=qn)
                            gather_names.append(gi_inst.ins.name)
                            qctr[0] += 1

                def s_build():
                    # S = (iota == dstrel) via pair-dup + int32-view expand
                    drfull = wpool.tile([P, GMAX, P], BF16, tag="drfull")
                    if dr_act:
                        nc.scalar.activation(
                            out=drfull[:, 0:gpb],
                            in_=dr_t[:, 0:gpb].unsqueeze(2).to_broadcast(
                                [P, gpb, P]),
                            func=mybir.ActivationFunctionType.Copy)
                    else:
                        drdup = smpool.tile([P, GMAX, 2], BF16, tag="drdup")
                        nc.vector.tensor_copy(
                            out=drdup[:, 0:gpb],
                            in_=dr_t[:, 0:gpb].unsqueeze(2).to_broadcast(
                                [P, gpb, 2]))
                        nc.vector.tensor_copy(
                            out=drfull[:, 0:gpb].bitcast(I32),
                            in_=drdup[:, 0:gpb].bitcast(I32).to_broadcast(
                                [P, gpb, P // 2]))
                    S_t = wpool.tile([P, GMAX, P], BF16, tag="S")
                    nc.vector.tensor_tensor(
                        out=S_t[:, 0:gpb],
                        in0=iota_c[:].unsqueeze(1).to_broadcast([P, gpb, P]),
                        in1=drfull[:, 0:gpb], op=A.is_equal)
                    return S_t

                # z~ = xl~ + xr~ (in-place into zb; raw xr~ is dead after the
                # add: the message side uses raw za only). Prelu lands in zw,
                # which is later overwritten by the wfull expansion (the tree
                # has consumed it by then).
                zw = wpool.tile([P, GMAX, D], BF16, tag="wfull")
                if pe_add:
                    # PE identity-matmul add into f32 PSUM chunks; ACT Prelu
                    # drains each chunk into zw
                    CH = 512
                    zaf = za[:, 0:gpb].rearrange("p g d -> p (g d)")
                    zbf = zb[:, 0:gpb].rearrange("p g d -> p (g d)")
                    zwf = zw[:, 0:gpb].rearrange("p g d -> p (g d)")
                    tot = gpb * D
                    for c0 in range(0, tot, CH):
                        w = min(CH, tot - c0)
                        zps = pszpool.tile([P, CH], F32, tag="zadd")
                        nc.tensor.matmul(out=zps[:, 0:w], lhsT=ident_c[:],
                                         rhs=zaf[:, c0:c0 + w],
                                         start=True, stop=False)
                        nc.tensor.matmul(out=zps[:, 0:w], lhsT=ident_c[:],
                                         rhs=zbf[:, c0:c0 + w],
                                         start=False, stop=True)
                        nc.scalar.activation(
                            out=zwf[:, c0:c0 + w], in_=zps[:, 0:w],
                            func=mybir.ActivationFunctionType.Prelu,
                            alpha=alpha_c[:, :])
                else:
                    nc.vector.tensor_tensor(out=zb[:, 0:gpb],
                                            in0=za[:, 0:gpb],
                                            in1=zb[:, 0:gpb], op=A.add)
                    nc.scalar.activation(
                        out=(zb if inplace_act else zw)[:, 0:gpb],
                        in_=zb[:, 0:gpb],
                        func=mybir.ActivationFunctionType.Prelu,
                        alpha=alpha_c[:, :])
                    if inplace_act:
                        zw = zb
                # S build emitted here fills the DVE bubble while the ACT
                # engine runs the Prelu
                S_t = s_build() if s_early else None
                # sign fix: negate each head's negative-att column block
                zbh = zw[:].rearrange("p g (h c) -> p g h c", h=H)
                for h in range(H):
                    ph = pcounts[h]
                    if ph < C:
                        nc.vector.tensor_scalar_mul(
                            out=zbh[:, 0:gpb, h, ph:C],
                            in0=zbh[:, 0:gpb, h, ph:C], scalar1=-1.0)
                # tree-reduce over c (2x-mode adds; last step to fp32)
                e16 = smpool.tile([P, GMAX, H, 16], BF16, tag="e16")
                nc.vector.tensor_tensor(out=e16[:, 0:gpb],
                                        in0=zbh[:, 0:gpb, :, 0:16],
                                        in1=zbh[:, 0:gpb, :, 16:32], op=A.add)
                e8 = smpool.tile([P, GMAX, H, 8], BF16, tag="e8")
                nc.vector.tensor_tensor(out=e8[:, 0:gpb],
                                        in0=e16[:, 0:gpb, :, 0:8],
                                        in1=e16[:, 0:gpb, :, 8:16], op=A.add)
                e4 = smpool.tile([P, GMAX, H, 4], BF16, tag="e4")
                nc.vector.tensor_tensor(out=e4[:, 0:gpb],
                                        in0=e8[:, 0:gpb, :, 0:4],
                                        in1=e8[:, 0:gpb, :, 4:8], op=A.add)
                e2 = smpool.tile([P, GMAX, H, 2], BF16, tag="e2")
                nc.vector.tensor_tensor(out=e2[:, 0:gpb],
                                        in0=e4[:, 0:gpb, :, 0:2],
                                        in1=e4[:, 0:gpb, :, 2:4], op=A.add)
                e1 = smpool.tile([P, GMAX, H], F32, tag="e1")
                nc.vector.tensor_tensor(out=e1[:, 0:gpb],
                                        in0=e2[:, 0:gpb, :, 0],
                                        in1=e2[:, 0:gpb, :, 1], op=A.add)
                # w = exp(e)
                w_t = smpool.tile([P, GMAX, H], BF16, tag="w")
                nc.scalar.activation(out=w_t[:, 0:gpb], in_=e1[:, 0:gpb],
                                     func=mybir.ActivationFunctionType.Exp)
                # expand w to [P,G,H,C] via pair-dup + int32-view copy
                wdup = smpool.tile([P, GMAX, H, 2], BF16, tag="wdup")
                nc.vector.tensor_copy(
                    out=wdup[:, 0:gpb],
                    in_=w_t[:, 0:gpb].unsqueeze(3).to_broadcast(
                        [P, gpb, H, 2]))
                wfull = zw if not inplace_act else wpool.tile(
                    [P, GMAX, D], BF16, tag="wf2")
                nc.vector.tensor_copy(
                    out=wfull[:, 0:gpb].bitcast(I32).rearrange(
                        "p g (h c) -> p g h c", h=H),
                    in_=wdup[:, 0:gpb].bitcast(I32).to_broadcast(
                        [P, gpb, H, C // 2]))
                # rhs = [wfull * xl_raw | w]
                rhs = wpool.tile([P, GMAX, D + H], BF16, tag="rhs")
                nc.vector.tensor_tensor(out=rhs[:, 0:gpb, 0:D],
                                        in0=za[:, 0:gpb], in1=wfull[:, 0:gpb],
                                        op=A.mult)
                nc.vector.tensor_copy(out=rhs[:, 0:gpb, D:D + H],
                                      in_=w_t[:, 0:gpb])
                if not s_early:
                    S_t = s_build()
                return (b, gpb, S_t, rhs)

            def back(st, bias_c, out_rows, interlude=None):
                b, gpb, S_t, rhs = st
                ps = pspool.tile([P, D + H], F32, tag="agg")
                for gi in range(gpb):
                    nc.tensor.matmul(
                        out=ps[:], lhsT=S_t[:, gi, :], rhs=rhs[:, gi, :],
                        start=(gi == 0), stop=(gi == gpb - 1))

                # denom > 0 always: every node has a self-loop edge
                dinv = smpool.tile([P, H], F32, tag="dinv")
                nc.vector.reciprocal(out=dinv[:], in_=ps[:, D:D + H])
                t3 = smpool.tile([P, D], F32, tag="t3")
                nc.vector.tensor_tensor(
                    out=t3[:].rearrange("p (h c) -> p h c", h=H),
                    in0=ps[:, 0:D].rearrange("p (h c) -> p h c", h=H),
                    in1=dinv[:].unsqueeze(2).to_broadcast([P, H, C]),
                    op=A.mult)
                t4 = smpool.tile([P, D], F32, tag="t4")
                nc.vector.tensor_tensor(out=t4[:], in0=t3[:], in1=bias_c[:],
                                        op=A.add)
                hrow = smpool.tile([P, D], BF16, tag="hrow")
                nc.vector.tensor_scalar_max(out=hrow[:], in0=t4[:],
                                            scalar1=0.0)
                nc.sync.dma_start(out=out_rows[b * P:(b + 1) * P, :],
                                  in_=hrow[:])
                if interlude is not None:
                    interlude(b)

            def interlude(b):
                htile = smpool.tile([P, D], BF16, tag="pl_h")
                nc.sync.dma_start(out=htile[:],
                                  in_=h1_loc[b * P:(b + 1) * P, :])
                psT = ps2pool.tile([P, P], BF16, tag="pl_T")
                nc.tensor.transpose(out=psT[:], in_=htile[:],
                                    identity=ident_c[:])
                hT = smpool.tile([P, P], BF16, tag="pl_hT")
                nc.vector.tensor_copy(out=hT[:], in_=psT[:])
                for W_c, table in ((W2l_c, xl2_slab), (W2r_c, xr2_loc)):
                    psm = ps2pool.tile([P, D], F32, tag="pl_mm")
                    nc.tensor.matmul(out=psm[:], lhsT=hT[:], rhs=W_c[:],
                                     start=True, stop=True)
                    res = smpool.tile([P, D], BF16, tag="pl_res")
                    nc.vector.tensor_copy(out=res[:], in_=psm[:])
                    nc.sync.dma_start(out=table[b * P:(b + 1) * P, :],
                                      in_=res[:])
                # chunked AllGather: fire chunk c once its last block's
                # interlude has written xl2_slab rows
                for c, (b0, b1) in enumerate(bounds):
                    if b == b1 - 1:
                        nc.gpsimd.collective_compute(
                            "AllGather", A.bypass,
                            replica_groups=[list(range(pl.NC))],
                            ins=[xl2_slab[b0 * P:b1 * P, :].opt()],
                            outs=[xl2_full[bases[c]:
                                           bases[c] + pl.NC * (b1 - b0) * P,
                                           :].opt()],
                        )

            def layer(G_lo, G_hi, tab_lo, tab_hi, tab_B, blkidx, dstrel_p,
                      pcounts, bias_c, out_rows, inter):
                pend = None
                for b in range(NBLK):
                    st = front(b, G_lo, G_hi, tab_lo, tab_hi, tab_B, blkidx,
                               dstrel_p, pcounts)
                    if not skew:
                        back(st, bias_c, out_rows, inter)
                        continue
                    if pend is not None:
                        back(pend, bias_c, out_rows, inter)
                    pend = st
                if pend is not None:
                    back(pend, bias_c, out_rows, inter)

            for _rep in range(repeat):
                layer(pl.G1_lo, pl.G1_hi, xl1[0:NLO1, :], xl1[NLO1:pl.N, :],
                      xr1_loc[:, :], blkidx_l1, dstrel1_p, pcounts1, bias1_c,
                      h1_loc, interlude)
                layer(pl.G2_lo, pl.G2_hi, xl2_full[0:SPLIT2, :],
                      xl2_full[SPLIT2:pl.NC * SLAB, :], xr2_loc[:, :],
                      blkidx_l2, dstrel2_p, pcounts2, bias2_c, out_p, None)
    nc._gather_names = gather_names
    return nc


def _scheduled_swdge_order(nc):
    """Pool-engine DMA instruction names in SCHEDULED order (= DMASW sem
    lane assignment order)."""
    import concourse.bass_isa as bass_isa
    out = []
    for blk in nc.m.functions[0].blocks:
        for inst in blk.instructions:
            if (getattr(inst, 'engine', None) == mybir.EngineType.Pool
                    and isinstance(inst, bass_isa.AnyDMAInstruction)):
                out.append(inst.name)
    return out


def build_kernel_auto(plan, pcounts1, pcounts2, repeat=1, tries=4, **kw):
    """Two-pass queue assignment: compile, read the scheduled SWDGE order,
    rebuild with queue = scheduled-position % 4 so each of the 8 DMASW sem
    lanes (assigned in scheduled order) sees exactly one queue. Iterate in
    case the queue change perturbs the schedule; fall back to single queue."""
    kw.pop('qmode', None)
    qmode = 'single'
    nc_single = None
    for t in range(tries):
        nc = build_kernel(plan, pcounts1, pcounts2, repeat=repeat,
                          qmode=qmode, **kw)
        nc.compile()
        order = _scheduled_swdge_order(nc)
        pos = {n: i for i, n in enumerate(order)}
        em = nc._gather_names
        lanes = {}
        ok = True
        for ei, n in enumerate(em):
            lane = pos[n] % 8
            q = qmode.get(ei, 0) if isinstance(qmode, dict) else 0
            if lanes.setdefault(lane, q) != q:
                ok = False
        if ok and isinstance(qmode, dict):
            return nc
        if not isinstance(qmode, dict):
            nc_single = nc
        qmode = {ei: pos[n] % 4 for ei, n in enumerate(em)}
    if nc_single is None:
        nc_single = build_kernel(plan, pcounts1, pcounts2, repeat=repeat,
                                 qmode='single', **kw)
        nc_single.compile()
    return nc_single


def make_host_tables(x, W1_l, W1_r, att1, b1, W2_l, W2_r, att2, b2):
    """Host-side scaled/permuted tables and constants."""
    perm1, s1, pc1 = sign_perm(att1)
    perm2, s2, pc2 = sign_perm(att2)
    x = np.asarray(x, np.float32)
    xl1 = (x @ np.asarray(W1_l, np.float32))[:, perm1] * s1
    xr1 = (x @ np.asarray(W1_r, np.float32))[:, perm1] * s1
    # W2': rows in perm1 order, unscaled by 1/s1; cols in perm2 order, scaled
    W2l = (np.asarray(W2_l, np.float32)[perm1][:, perm2] * s2) / s1[:, None]
    W2r = (np.asarray(W2_r, np.float32)[perm1][:, perm2] * s2) / s1[:, None]
    bias1 = np.asarray(b1, np.float32)[perm1] * s1
    bias2 = np.asarray(b2, np.float32)[perm2] * s2
    return dict(perm1=perm1, s1=s1, pc1=pc1, perm2=perm2, s2=s2, pc2=pc2,
                xl1=xl1.astype(NPBF), xr1=xr1.astype(NPBF),
                W2l=W2l.astype(NPBF), W2r=W2r.astype(NPBF),
                bias1=np.tile(bias1[None, :], (P, 1)).astype(np.float32),
                bias2=np.tile(bias2[None, :], (P, 1)).astype(np.float32))


def make_inputs(plan, datas, ht):
    pl = plan
    iota = np.tile(np.arange(P, dtype=np.float32)[None, :], (P, 1)).astype(NPBF)
    ident = np.eye(P, dtype=np.float32).astype(NPBF)

    in_maps = []
    for k in range(pl.NC):
        xr1_loc = np.zeros((pl.SLAB, D), NPBF)
        nreal = min(pl.NPC, pl.N - k * pl.NPC)
        xr1_loc[:nreal] = ht['xr1'][k * pl.NPC: k * pl.NPC + nreal]
        in_maps.append(dict(
            xl1=ht['xl1'],
            xr1_loc=xr1_loc,
            blkidx_l1=datas[k]["blkidx_l1"],
            blkidx_l2=datas[k]["blkidx_l2"],
            dstrel1=datas[k]["dstrel1"],
            dstrel2=datas[k]["dstrel2"],
            iota=iota, ident=ident,
            W2l=ht['W2l'], W2r=ht['W2r'],
            bias1=ht['bias1'], bias2=ht['bias2'],
        ))
    return in_maps


def assemble_output(plan, results, ht):
    out = np.zeros((plan.N, D), np.float32)
    for k in range(plan.NC):
        out[k * plan.NPC:(k + 1) * plan.NPC] = \
            results[k]["out"][:plan.NPC].astype(np.float32)
    # undo layer-2 column scale+permutation
    full = np.empty_like(out)
    full[:, ht['perm2']] = out / ht['s2'][None, :]
    return full


def kernel(x, edge_index, W1_l, W1_r, att1, b1, W2_l, W2_r, att2, b2):
    x = np.ascontiguousarray(np.asarray(x, np.float32))
    edge_index = np.asarray(edge_index)
    plan, datas = preprocess(x, edge_index, NC=8)
    ht = make_host_tables(x, W1_l, W1_r, att1, b1, W2_l, W2_r, att2, b2)
    nc = build_kernel_auto(plan, ht['pc1'], ht['pc2'])
    in_maps = make_inputs(plan, datas, ht)
    res = run_bass_kernel_spmd(nc, in_maps, core_ids=list(range(8)))
    return assemble_output(plan, res.results, ht)
